# revision 84
# baseline (speedup 1.0000x reference)
"""NetVLAD forward on 8 Trainium2 NeuronCores.

Reference computation (per batch b):
    logits = conv_w @ x_flat[b]            # [K, N]    (K=64, C=128, N=4096)
    a      = softmax(logits, axis=K)
    vlad   = a @ x_flat[b].T - sum_n(a) * centroids    # [K, C]
    vlad   = l2norm(vlad, axis=C); out[b] = l2norm(vlad.reshape(K*C))

Sharding: data-parallel over batch (8 per core), weights replicated.

Device computes per batch the raw [vladT | asum] = [[sum_n a x^T]^T, sum_n a]
as a [C, K+1] psum tile; the tiny epilogue (centroid subtraction + two L2
norms, <1% FLOPs) runs on the host after the gather.

Key structure (driven by the TimelineSim cost model, where a matmul costs
out-free-size rows and stationary loads are free):
  - mm1  (PE): logits[n,K]   = x_chunk^T @ w          -> 64 rows/chunk
  - T    (PE): xT[n,C]       = transpose(x_chunk)      -> 128 rows/chunk,
               into psum pt[n, g, c] (g-major: PSUM writes must be 4B aligned)
  - ACT: e = exp(logits) psum->sbuf (bf16, k-minor)
  - DVE: s = sum_k e (bf16 2x), r2 = 1/s duplicated as [n, g, 2] pairs
  - DVE/Pool: xts[n,g,c] = pt * r[n,g]  (fused copy+scale). The broadcast
    of r over c would normally put a stride-0 innermost dim on in1 and lose
    the DVE 2x perf mode; instead all three operands use a 4-dim AP with
    innermost packed pair [1,2] (out/in0 split c into c/2 x 2; in1 reads the
    duplicated r2 pair) so 2x-1port survives.
  - mm2' (PE): vladT[C,K] += xts_chunk^T @ e_chunk     -> 64 rows/chunk
  - asum (PE): pv[0:64,64] += e_chunk^T @ r_col        -> 1 row/chunk
One-batch software pipelining keeps the PE continuously busy (p-state ramp
to 2.4 GHz after 3us).

Softmax skips max-subtraction: logits ~ N(0, 1.3), |logit| < 8 for this
input distribution, exp() stays comfortably in range.
"""

import numpy as np
import ml_dtypes
from contextlib import ExitStack

import concourse.bass as bass
import concourse.bacc as bacc
import concourse.tile as tile
import concourse.mybir as mybir
from concourse import bass_utils

B, C, K = 64, 128, 64
HW = 64 * 64  # N = H*W
NCORES = 8
BPC = B // NCORES  # batches per core
F32 = mybir.dt.float32
BF16 = mybir.dt.bfloat16

NCHUNK = 128            # n-columns per chunk (PE contraction limit)
NCH = HW // NCHUNK      # chunks per batch = 32
G = 8                   # chunks per group (one full 2KB psum bank)
NG = NCH // G           # groups per batch = 4

# per-group handling of the psum->sbuf xT copy and the softmax scale:
#   "dve":      DVE does a fused copy+scale (2x perf mode via the pair trick)
#   "act_pool": ACT does a plain copy; Pool scales e->a in SBUF (GPSIMD
#               cannot touch PSUM), and mm2'/asum use a with a ones column
COPYSCALE = ("dve", "dve", "act_pool", "dve")


def _pair_view(t_ap):
    """[128, G, C] AP -> [128, G, C/2, 2] (same memory, innermost packed pair)."""
    p, g, c = t_ap.ap
    return bass.AP(
        tensor=t_ap.tensor,
        offset=t_ap.offset,
        ap=[p, g, [2 * c[0], c[1] // 2], [c[0], 2]],
    )


def _r2_bcast(r2_ap, n_c):
    """r2 [128, G, 2] AP -> [128, G, n_c/2, 2]: broadcast the duplicated pair
    over c via a stride-0 middle dim, keeping the innermost dim packed so the
    DVE 2x perf mode survives."""
    p, g, two = r2_ap.ap
    return bass.AP(
        tensor=r2_ap.tensor,
        offset=r2_ap.offset,
        ap=[p, g, [0, n_c // 2], two],
    )


def _dup2(r_ap):
    """r [128, M] AP -> [128, M, 2] input view repeating each value twice."""
    p, m = r_ap.ap
    return bass.AP(tensor=r_ap.tensor, offset=r_ap.offset, ap=[p, m, [0, 2]])


def _netvlad_tile(tc: tile.TileContext, out_d, x_d, w_d, ident_d):
    nc = tc.nc
    with ExitStack() as ctx:
        const = ctx.enter_context(tc.tile_pool(name="const", bufs=1))
        xpool = ctx.enter_context(tc.tile_pool(name="x", bufs=4))
        epool = ctx.enter_context(tc.tile_pool(name="e", bufs=3))
        rpool = ctx.enter_context(tc.tile_pool(name="r", bufs=3))
        spool = ctx.enter_context(tc.tile_pool(name="s", bufs=2))
        xtspool = ctx.enter_context(tc.tile_pool(name="xts", bufs=3 * NG))
        opool = ctx.enter_context(tc.tile_pool(name="o", bufs=BPC))
        pl_pool = ctx.enter_context(tc.tile_pool(name="pl", bufs=3, space="PSUM"))
        pt_pool = ctx.enter_context(tc.tile_pool(name="pt", bufs=3, space="PSUM"))
        pv_pool = ctx.enter_context(tc.tile_pool(name="pv", bufs=1, space="PSUM"))
        pa_pool = ctx.enter_context(tc.tile_pool(name="pa", bufs=1, space="PSUM"))

        w_sb = const.tile([C, K], BF16)
        nc.sync.dma_start(out=w_sb, in_=w_d)
        ident_sb = const.tile([C, C], BF16)
        ones_sb = const.tile([NCHUNK, 1], BF16)
        nc.gpsimd.memset(ones_sb, 1.0)

        xs_tiles = [None] * (BPC + 1)

        def load_x(ib):
            xt = xpool.tile([C, HW], BF16, tag="x")
            for q in range(4):
                lo, hi = q * HW // 4, (q + 1) * HW // 4
                nc.sync.dma_start(out=xt[:, lo:hi], in_=x_d[ib][:, lo:hi])
            xs_tiles[ib] = xt

        # state carried from phase1(ib) to phase2(ib)
        saved = [None] * BPC
        staged = []

        def phase1(ib):
            x = xs_tiles[ib]
            e = epool.tile([NCHUNK, NCH, K], BF16, tag="e")
            pts = []
            for g in range(NG):
                pl = pl_pool.tile([NCHUNK, G, K], F32)
                pt = pt_pool.tile([NCHUNK, G, C], BF16)
                for i in range(G):
                    ch = g * G + i
                    xsl = x[:, ch * NCHUNK : (ch + 1) * NCHUNK]
                    nc.tensor.matmul(
                        pl[:, i, :], lhsT=xsl, rhs=w_sb, start=True, stop=True
                    )
                    nc.tensor.transpose(pt[:, i, :], in_=xsl, identity=ident_sb)
                nc.scalar.activation(
                    e[:, g * G : (g + 1) * G, :],
                    pl,
                    mybir.ActivationFunctionType.Exp,
                )
                pts.append(pt)
                if g == 0:
                    s = spool.tile([NCHUNK, NCH], F32, tag="s")
                if g % 2 == 1:
                    lo = (g - 1) * G
                    nc.vector.reduce_sum(
                        s[:, lo : lo + 2 * G],
                        e[:, lo : lo + 2 * G, :],
                        axis=mybir.AxisListType.X,
                    )
            # batch-wide reciprocal straight into duplicated bf16 pairs
            r2 = rpool.tile([NCHUNK, NCH, 2], BF16, tag="r")
            nc.vector.reciprocal(r2, _dup2(s))
            a = epool.tile([NCHUNK, A_CH, K], BF16, tag="a")
            r_b = bass.AP(
                tensor=r2.tensor,
                offset=r2.offset,
                ap=[r2.ap[0], [2, A_CH], [0, K]],
            )
            nc.gpsimd.tensor_tensor(
                out=a, in0=e[:, 0:A_CH, :], in1=r_b, op=mybir.AluOpType.mult
            )
            xtss = []
            for g in range(NG):
                pt = pts[g]
                lo = g * G
                xts = xtspool.tile([NCHUNK, G, C], BF16, tag="xts")
                n_act = max(0, min(A_CH - lo, G))
                if n_act > 0:
                    nc.scalar.copy(
                        out=xts[:, 0:n_act, :], in_=pt[:, 0:n_act, :]
                    )
                if n_act < G:
                    nc.vector.tensor_tensor(
                        out=_pair_view(xts[:, n_act:G, :]),
                        in0=_pair_view(pt[:, n_act:G, :]),
                        in1=_r2_bcast(r2[:, lo + n_act : lo + G, :], C),
                        op=mybir.AluOpType.mult,
                    )
                xtss.append(xts)
            saved[ib] = (e, a, r2, xtss)

        def phase2(ib):
            e, a, r2, xtss = saved[ib]
            pv = pv_pool.tile([C, K], F32)  # vladT
            pa = pa_pool.tile([K, 1], F32)  # asum
            for g in range(NG):
                xts = xtss[g]
                for i in range(G):
                    ch = g * G + i
                    scaled_x = ch >= A_CH
                    mv_sl = e[:, ch, :] if scaled_x else a[:, ch, :]
                    nc.tensor.matmul(
                        pv,
                        lhsT=xts[:, i, :],
                        rhs=mv_sl,
                        start=(ch == 0),
                        stop=(ch == NCH - 1),
                    )
                    nc.tensor.matmul(
                        pa,
                        lhsT=mv_sl,
                        rhs=r2[:, ch, 0:1] if scaled_x else ones_sb,
                        start=(ch == 0),
                        stop=(ch == NCH - 1),
                    )
            outt = opool.tile([C, K], F32, tag="o")
            nc.scalar.copy(out=outt, in_=pv)
            outa = opool.tile([K, 1], F32, tag="oa")
            nc.scalar.copy(out=outa, in_=pa)
            staged.append((ib, outt, outa))
            saved[ib] = None

        with nc.allow_low_precision("softmax sums/recip in bf16: 0.4% rel err"):
            load_x(0)
            nc.sync.dma_start(out=ident_sb, in_=ident_d)
            load_x(1)
            load_x(2)
            for ib in range(BPC):
                phase1(ib)
                if ib + 3 < BPC:
                    load_x(ib + 3)
                if ib >= 1:
                    phase2(ib - 1)
            phase2(BPC - 1)
            out_v_d, out_a_d = out_d
            for ib, outt, outa in staged:
                nc.sync.dma_start(out=out_v_d[ib], in_=outt)
                nc.sync.dma_start(out=out_a_d[ib], in_=outa)


_NC_CACHE = None


def _get_nc():
    global _NC_CACHE
    if _NC_CACHE is None:
        nc = bacc.Bacc(
            "TRN2",
            target_bir_lowering=False,
            debug=False,
            num_devices=NCORES,
        )
        x_d = nc.dram_tensor("x", [BPC, C, HW], BF16, kind="ExternalInput").ap()
        w_d = nc.dram_tensor("w_t", [C, K], BF16, kind="ExternalInput").ap()
        ident_d = nc.dram_tensor("ident", [C, C], BF16, kind="ExternalInput").ap()
        out_v_d = nc.dram_tensor("out_v", [BPC, C, K], F32, kind="ExternalOutput").ap()
        out_a_d = nc.dram_tensor("out_a", [BPC, K, 1], F32, kind="ExternalOutput").ap()
        with tile.TileContext(nc) as tc:
            _netvlad_tile(tc, (out_v_d, out_a_d), x_d, w_d, ident_d)
        nc.compile()
        _NC_CACHE = nc
    return _NC_CACHE


def _make_in_maps(x, conv_w):
    bf16 = ml_dtypes.bfloat16
    x_flat = np.ascontiguousarray(x.reshape(B, C, HW).astype(bf16))
    w_t = np.ascontiguousarray(conv_w.T.astype(bf16))  # [C, K]
    ident = np.eye(C, dtype=np.float32).astype(bf16)
    in_maps = []
    for core in range(NCORES):
        in_maps.append(
            {
                "x": x_flat[core * BPC : (core + 1) * BPC],
                "w_t": w_t,
                "ident": ident,
            }
        )
    return in_maps


def _run(in_maps, trace=False, **kwargs):
    nc = _get_nc()
    return bass_utils.run_bass_kernel_spmd(
        nc, in_maps, core_ids=list(range(NCORES)), trace=trace, **kwargs
    )


def _postprocess(raw_v, raw_a, centroids):
    """raw_v: [B, C, K] vladT; raw_a: [B, K] asum -> [B, K*C] normalized."""
    vlad = raw_v.transpose(0, 2, 1) - raw_a[:, :, None] * centroids
    norms = np.sqrt((vlad * vlad).sum(axis=2, keepdims=True))
    vlad = vlad / np.maximum(norms, 1e-12)
    out = vlad.reshape(raw_v.shape[0], K * C)
    gn = np.sqrt((out * out).sum(axis=1, keepdims=True))
    return out / np.maximum(gn, 1e-12)


def kernel(x, conv_w, centroids):
    x = np.asarray(x)
    conv_w = np.asarray(conv_w)
    centroids = np.asarray(centroids, dtype=np.float32)
    res = _run(_make_in_maps(x, conv_w))
    raw_v = np.concatenate([r["out_v"] for r in res.results], axis=0)  # [B, C, K]
    raw_a = np.concatenate([r["out_a"] for r in res.results], axis=0)[:, :, 0]
    return _postprocess(
        raw_v.astype(np.float32), raw_a.astype(np.float32), centroids
    ).astype(np.float32)


# revision 85
# speedup vs baseline: 1.0267x; 1.0267x over previous
"""NetVLAD forward on 8 Trainium2 NeuronCores.

Reference computation (per batch b):
    logits = conv_w @ x_flat[b]            # [K, N]    (K=64, C=128, N=4096)
    a      = softmax(logits, axis=K)
    vlad   = a @ x_flat[b].T - sum_n(a) * centroids    # [K, C]
    vlad   = l2norm(vlad, axis=C); out[b] = l2norm(vlad.reshape(K*C))

Sharding: data-parallel over batch (8 per core), weights replicated.

Device computes per batch the raw [vladT | asum] = [[sum_n a x^T]^T, sum_n a]
as a [C, K+1] psum tile; the tiny epilogue (centroid subtraction + two L2
norms, <1% FLOPs) runs on the host after the gather.

Key structure (driven by the TimelineSim cost model, where a matmul costs
out-free-size rows and stationary loads are free):
  - mm1  (PE): logits[n,K]   = x_chunk^T @ w          -> 64 rows/chunk
  - T    (PE): xT[n,C]       = transpose(x_chunk)      -> 128 rows/chunk,
               into psum pt[n, g, c] (g-major: PSUM writes must be 4B aligned)
  - ACT: e = exp(logits) psum->sbuf (bf16, k-minor)
  - DVE: s = sum_k e (bf16 2x), r2 = 1/s duplicated as [n, g, 2] pairs
  - DVE/Pool: xts[n,g,c] = pt * r[n,g]  (fused copy+scale). The broadcast
    of r over c would normally put a stride-0 innermost dim on in1 and lose
    the DVE 2x perf mode; instead all three operands use a 4-dim AP with
    innermost packed pair [1,2] (out/in0 split c into c/2 x 2; in1 reads the
    duplicated r2 pair) so 2x-1port survives.
  - mm2' (PE): vladT[C,K] += xts_chunk^T @ e_chunk     -> 64 rows/chunk
  - asum (PE): pv[0:64,64] += e_chunk^T @ r_col        -> 1 row/chunk
One-batch software pipelining keeps the PE continuously busy (p-state ramp
to 2.4 GHz after 3us).

Softmax skips max-subtraction: logits ~ N(0, 1.3), |logit| < 8 for this
input distribution, exp() stays comfortably in range.
"""

import numpy as np
import ml_dtypes
from contextlib import ExitStack

import concourse.bass as bass
import concourse.bacc as bacc
import concourse.tile as tile
import concourse.mybir as mybir
from concourse import bass_utils

B, C, K = 64, 128, 64
HW = 64 * 64  # N = H*W
NCORES = 8
BPC = B // NCORES  # batches per core
F32 = mybir.dt.float32
BF16 = mybir.dt.bfloat16

NCHUNK = 128            # n-columns per chunk (PE contraction limit)
NCH = HW // NCHUNK      # chunks per batch = 32
G = 8                   # chunks per group (one full 2KB psum bank)
NG = NCH // G           # groups per batch = 4

# per-group handling of the psum->sbuf xT copy and the softmax scale:
#   "dve":      DVE does a fused copy+scale (2x perf mode via the pair trick)
#   "act_pool": ACT does a plain copy; Pool scales e->a in SBUF (GPSIMD
#               cannot touch PSUM), and mm2'/asum use a with a ones column
COPYSCALE = ("dve", "dve", "act_pool", "dve")


def _pair_view(t_ap):
    """[128, G, C] AP -> [128, G, C/2, 2] (same memory, innermost packed pair)."""
    p, g, c = t_ap.ap
    return bass.AP(
        tensor=t_ap.tensor,
        offset=t_ap.offset,
        ap=[p, g, [2 * c[0], c[1] // 2], [c[0], 2]],
    )


def _r2_bcast(r2_ap, n_c):
    """r2 [128, G, 2] AP -> [128, G, n_c/2, 2]: broadcast the duplicated pair
    over c via a stride-0 middle dim, keeping the innermost dim packed so the
    DVE 2x perf mode survives."""
    p, g, two = r2_ap.ap
    return bass.AP(
        tensor=r2_ap.tensor,
        offset=r2_ap.offset,
        ap=[p, g, [0, n_c // 2], two],
    )


def _dup2(r_ap):
    """r [128, M] AP -> [128, M, 2] input view repeating each value twice."""
    p, m = r_ap.ap
    return bass.AP(tensor=r_ap.tensor, offset=r_ap.offset, ap=[p, m, [0, 2]])


def _netvlad_tile(tc: tile.TileContext, out_d, x_d, w_d, ident_d):
    nc = tc.nc
    with ExitStack() as ctx:
        const = ctx.enter_context(tc.tile_pool(name="const", bufs=1))
        xpool = ctx.enter_context(tc.tile_pool(name="x", bufs=4))
        epool = ctx.enter_context(tc.tile_pool(name="e", bufs=3))
        rpool = ctx.enter_context(tc.tile_pool(name="r", bufs=3))
        spool = ctx.enter_context(tc.tile_pool(name="s", bufs=2))
        xtspool = ctx.enter_context(tc.tile_pool(name="xts", bufs=3 * NG))
        opool = ctx.enter_context(tc.tile_pool(name="o", bufs=BPC))
        pl_pool = ctx.enter_context(tc.tile_pool(name="pl", bufs=3, space="PSUM"))
        pt_pool = ctx.enter_context(tc.tile_pool(name="pt", bufs=3, space="PSUM"))
        pv_pool = ctx.enter_context(tc.tile_pool(name="pv", bufs=1, space="PSUM"))
        pa_pool = ctx.enter_context(tc.tile_pool(name="pa", bufs=1, space="PSUM"))

        w_sb = const.tile([C, K], BF16)
        nc.sync.dma_start(out=w_sb, in_=w_d)
        ident_sb = const.tile([C, C], BF16)
        ones_sb = const.tile([NCHUNK, 1], BF16)
        nc.gpsimd.memset(ones_sb, 1.0)

        xs_tiles = [None] * (BPC + 1)

        def load_x(ib):
            xt = xpool.tile([C, HW], BF16, tag="x")
            for q in range(4):
                lo, hi = q * HW // 4, (q + 1) * HW // 4
                nc.sync.dma_start(out=xt[:, lo:hi], in_=x_d[ib][:, lo:hi])
            xs_tiles[ib] = xt

        # state carried from phase1(ib) to phase2(ib)
        saved = [None] * BPC
        staged = []

        def phase1(ib):
            x = xs_tiles[ib]
            e = epool.tile([NCHUNK, NCH, K], BF16, tag="e")
            pts = []
            for g in range(NG):
                pl = pl_pool.tile([NCHUNK, G, K], F32)
                pt = pt_pool.tile([NCHUNK, G, C], BF16)
                for i in range(G):
                    ch = g * G + i
                    xsl = x[:, ch * NCHUNK : (ch + 1) * NCHUNK]
                    nc.tensor.matmul(
                        pl[:, i, :], lhsT=xsl, rhs=w_sb, start=True, stop=True
                    )
                    nc.tensor.transpose(pt[:, i, :], in_=xsl, identity=ident_sb)
                nc.scalar.activation(
                    e[:, g * G : (g + 1) * G, :],
                    pl,
                    mybir.ActivationFunctionType.Exp,
                )
                pts.append(pt)
                if g == 0:
                    s = spool.tile([NCHUNK, NCH], F32, tag="s")
                nc.vector.reduce_sum(
                    s[:, g * G : (g + 1) * G],
                    e[:, g * G : (g + 1) * G, :],
                    axis=mybir.AxisListType.X,
                )
            # batch-wide reciprocal straight into duplicated bf16 pairs
            r2 = rpool.tile([NCHUNK, NCH, 2], BF16, tag="r")
            nc.vector.reciprocal(r2, _dup2(s))
            a = epool.tile([NCHUNK, A_CH, K], BF16, tag="a")
            r_b = bass.AP(
                tensor=r2.tensor,
                offset=r2.offset,
                ap=[r2.ap[0], [2, A_CH], [0, K]],
            )
            nc.gpsimd.tensor_tensor(
                out=a, in0=e[:, 0:A_CH, :], in1=r_b, op=mybir.AluOpType.mult
            )
            xtss = []
            for g in range(NG):
                pt = pts[g]
                lo = g * G
                xts = xtspool.tile([NCHUNK, G, C], BF16, tag="xts")
                n_act = max(0, min(A_CH - lo, G))
                if n_act > 0:
                    nc.scalar.copy(
                        out=xts[:, 0:n_act, :], in_=pt[:, 0:n_act, :]
                    )
                if n_act < G:
                    nc.vector.tensor_tensor(
                        out=_pair_view(xts[:, n_act:G, :]),
                        in0=_pair_view(pt[:, n_act:G, :]),
                        in1=_r2_bcast(r2[:, lo + n_act : lo + G, :], C),
                        op=mybir.AluOpType.mult,
                    )
                xtss.append(xts)
            saved[ib] = (e, a, r2, xtss)

        def phase2(ib):
            e, a, r2, xtss = saved[ib]
            pv = pv_pool.tile([C, K], F32)  # vladT
            pa = pa_pool.tile([K, 1], F32)  # asum
            for g in range(NG):
                xts = xtss[g]
                for i in range(G):
                    ch = g * G + i
                    scaled_x = ch >= A_CH
                    mv_sl = e[:, ch, :] if scaled_x else a[:, ch, :]
                    nc.tensor.matmul(
                        pv,
                        lhsT=xts[:, i, :],
                        rhs=mv_sl,
                        start=(ch == 0),
                        stop=(ch == NCH - 1),
                    )
                    nc.tensor.matmul(
                        pa,
                        lhsT=mv_sl,
                        rhs=r2[:, ch, 0:1] if scaled_x else ones_sb,
                        start=(ch == 0),
                        stop=(ch == NCH - 1),
                    )
            outt = opool.tile([C, K], F32, tag="o")
            nc.scalar.copy(out=outt, in_=pv)
            outa = opool.tile([K, 1], F32, tag="oa")
            nc.scalar.copy(out=outa, in_=pa)
            staged.append((ib, outt, outa))
            saved[ib] = None

        with nc.allow_low_precision("softmax sums/recip in bf16: 0.4% rel err"):
            load_x(0)
            nc.sync.dma_start(out=ident_sb, in_=ident_d)
            load_x(1)
            load_x(2)
            for ib in range(BPC):
                phase1(ib)
                if ib + 3 < BPC:
                    load_x(ib + 3)
                if ib >= 1:
                    phase2(ib - 1)
            phase2(BPC - 1)
            out_v_d, out_a_d = out_d
            for ib, outt, outa in staged:
                nc.sync.dma_start(out=out_v_d[ib], in_=outt)
                nc.sync.dma_start(out=out_a_d[ib], in_=outa)


_NC_CACHE = None


def _get_nc():
    global _NC_CACHE
    if _NC_CACHE is None:
        nc = bacc.Bacc(
            "TRN2",
            target_bir_lowering=False,
            debug=False,
            num_devices=NCORES,
        )
        x_d = nc.dram_tensor("x", [BPC, C, HW], BF16, kind="ExternalInput").ap()
        w_d = nc.dram_tensor("w_t", [C, K], BF16, kind="ExternalInput").ap()
        ident_d = nc.dram_tensor("ident", [C, C], BF16, kind="ExternalInput").ap()
        out_v_d = nc.dram_tensor("out_v", [BPC, C, K], F32, kind="ExternalOutput").ap()
        out_a_d = nc.dram_tensor("out_a", [BPC, K, 1], F32, kind="ExternalOutput").ap()
        with tile.TileContext(nc) as tc:
            _netvlad_tile(tc, (out_v_d, out_a_d), x_d, w_d, ident_d)
        nc.compile()
        _NC_CACHE = nc
    return _NC_CACHE


def _make_in_maps(x, conv_w):
    bf16 = ml_dtypes.bfloat16
    x_flat = np.ascontiguousarray(x.reshape(B, C, HW).astype(bf16))
    w_t = np.ascontiguousarray(conv_w.T.astype(bf16))  # [C, K]
    ident = np.eye(C, dtype=np.float32).astype(bf16)
    in_maps = []
    for core in range(NCORES):
        in_maps.append(
            {
                "x": x_flat[core * BPC : (core + 1) * BPC],
                "w_t": w_t,
                "ident": ident,
            }
        )
    return in_maps


def _run(in_maps, trace=False, **kwargs):
    nc = _get_nc()
    return bass_utils.run_bass_kernel_spmd(
        nc, in_maps, core_ids=list(range(NCORES)), trace=trace, **kwargs
    )


def _postprocess(raw_v, raw_a, centroids):
    """raw_v: [B, C, K] vladT; raw_a: [B, K] asum -> [B, K*C] normalized."""
    vlad = raw_v.transpose(0, 2, 1) - raw_a[:, :, None] * centroids
    norms = np.sqrt((vlad * vlad).sum(axis=2, keepdims=True))
    vlad = vlad / np.maximum(norms, 1e-12)
    out = vlad.reshape(raw_v.shape[0], K * C)
    gn = np.sqrt((out * out).sum(axis=1, keepdims=True))
    return out / np.maximum(gn, 1e-12)


def kernel(x, conv_w, centroids):
    x = np.asarray(x)
    conv_w = np.asarray(conv_w)
    centroids = np.asarray(centroids, dtype=np.float32)
    res = _run(_make_in_maps(x, conv_w))
    raw_v = np.concatenate([r["out_v"] for r in res.results], axis=0)  # [B, C, K]
    raw_a = np.concatenate([r["out_a"] for r in res.results], axis=0)[:, :, 0]
    return _postprocess(
        raw_v.astype(np.float32), raw_a.astype(np.float32), centroids
    ).astype(np.float32)
